# revision 67
# baseline (speedup 1.0000x reference)
"""GAT 2-layer GNN on 8 Trainium2 NeuronCores (Bass/Tile). Dev module (v4).

Sharding: nodes partitioned across 8 cores by dst ownership, assigned
round-robin over the global degree sort so every core's 128-node groups have
near-identical max in-degree (tight shared slot schedule). Per core, owned
nodes are processed in 128-node groups: node-on-partition layout, incoming-
edge slots along the free axis. The int16 dma_gather index limit forces two
overlapping source tables (rows [0,32768) and [NROWS-32768,NROWS)); edges
sourced in the overlap (cores 3-4) are assigned to whichever side balances
each dst node's slot counts. Padding slots gather a sentinel row (zero
features, el=-150) instead of using an additive mask. Feature tables are
bf16 (512B gather rows at layer 1); layer-1 per-head blocks live in a
rotated basis whose first two coordinates are el/er, so one gather row per
edge carries features + logits; the aggregate is rotated back by a bf16
matmul on the TensorEngine. Attention exp is widened to only 16 columns per
head on the Activation engine; the message multiply replicates each 16-wide
block across the head's 64 columns with a stride-0 AP dim, keeping DVE 2x
packed-bf16 rate. The layer-1 -> layer-2 halo exchange is a single bf16
AllGather collective; projection runs in group pairs to halve HWDGE
dispatches.
"""
import sys

sys.path.insert(0, "/opt/trn_rl_repo")
import numpy as np

import concourse.bass as bass
import concourse.bacc as bacc
import concourse.mybir as mybir
import concourse.tile as tile
from concourse.bass_utils import run_bass_kernel_spmd
from concourse.masks import make_identity

F32 = mybir.dt.float32
BF16 = mybir.dt.bfloat16
I16 = mybir.dt.int16

NC = 8
P = 128
NEG = 0.2
CMAX = 48  # max slots per compute chunk
SENT_EL = -150.0  # sentinel el (pre-leaky) for padding slots -> exp ~ 1e-13
WE = 16  # narrow exp width per head (replicated 4x by AP broadcast in msg mult)


class Cfg:
    def __init__(self, N, E, IN=256, H1=4, D1=64, O2=47):
        assert N % NC == 0
        self.N, self.E, self.IN, self.H1, self.D1, self.O2 = N, E, IN, H1, D1, O2
        assert H1 * D1 == IN
        self.NPC = N // NC
        self.G = -(-self.NPC // P)
        self.NPAD = self.G * P
        self.NROWS = NC * self.NPAD
        self.LO_END = 32768  # lo table rows [0, 32768) covers cores 0-4
        self.HI_START = self.NROWS - 32768  # hi table covers cores 3-7
        # int16 gather indices: any row in [0, LO_END) is lo-addressable; any
        # row >= HI_START is hi-addressable (rel = row - HI_START < 32768).
        self.SENT_LO = self.NPC  # core 0 padding row (features zero, el=SENT_EL)
        self.SENT_HI = (NC - 1) * self.NPAD + self.NPC - self.HI_START
        self.C2 = 128  # bf16 rows must be a multiple of 256B for dma_gather


class Sched:
    pass


def preprocess(cfg, x, src, dst):
    """Per-core shards + shared schedule. dst must be sorted. Nodes are
    assigned to cores round-robin over the global (total-degree) sort so the
    shared slot schedule (max over cores) stays tight. The int16 gather limit
    (32768 rows) forces two overlapping source tables: lo = rows [0, 32768)
    (cores 0-4), hi = rows [NROWS-32768, NROWS) (cores 3-7). Edges sourced in
    cores 3-4 can use either table; they are assigned to balance each dst
    node's lo/hi slot counts, which tightens the per-group max. The index
    blob is a single [128, 8*SUMD] int16 array whose column offset for slot
    d of group g is 8*(Doff[g]+d)."""
    c = cfg
    src = src.astype(np.int64)
    dst = dst.astype(np.int64)
    deg = np.bincount(dst, minlength=c.N)
    # pass 1: global degree sort, round-robin core assignment
    gorder = np.argsort(-deg, kind="stable")  # descending degree
    core_of = np.empty(c.N, np.int64)
    rank_pos = np.empty(c.N, np.int64)  # provisional rank within core
    core_of[gorder] = np.arange(c.N) % NC
    rank_pos[gorder] = np.arange(c.N) // NC

    # balanced lo/hi side per edge: forced by source core except cores 3-4
    src_core = core_of[src]
    can_lo = src_core <= 4
    can_hi = src_core >= 3
    flex = can_lo & can_hi
    f_lo = np.bincount(dst[can_lo & ~can_hi], minlength=c.N)
    f_x = np.bincount(dst[flex], minlength=c.N)
    lo_n = np.clip((deg + 1) // 2, f_lo, f_lo + f_x)
    flex_idx = np.where(flex)[0]
    d_f = dst[flex_idx]  # sorted (dst is sorted)
    pos_in_node = np.arange(len(d_f)) - np.searchsorted(d_f, d_f, side="left")
    hi_edge = ~can_lo
    hi_edge[flex_idx[pos_in_node >= (lo_n - f_lo)[d_f]]] = True

    # per-core per-node lo/hi in-degrees
    l1row = np.empty(c.N, np.int64)
    core_data = []
    Ls = np.zeros((NC, c.G), np.int64)
    Hs = np.zeros((NC, c.G), np.int64)
    edge_core = core_of[dst]
    for ci in range(NC):
        owned = np.where(core_of == ci)[0]  # global node ids, NPC of them
        owned = owned[np.argsort(rank_pos[owned])]  # by provisional rank
        esel = np.where(edge_core == ci)[0]
        s = src[esel]
        d = dst[esel]
        hi = hi_edge[esel]
        # local node index by provisional rank
        loc = rank_pos[d]
        lo_deg = np.bincount(loc[~hi], minlength=c.NPC)
        hi_deg = np.bincount(loc[hi], minlength=c.NPC)
        # pass 2: within-core re-sort by (max,min) of lo/hi — stays in-core,
        # so the lo/hi split (by table half) is unaffected
        order = np.lexsort((np.minimum(lo_deg, hi_deg), np.maximum(lo_deg, hi_deg)))
        l1row[owned[order]] = ci * c.NPAD + np.arange(c.NPC)
        lo_pad = np.zeros(c.NPAD, np.int64)
        hi_pad = np.zeros(c.NPAD, np.int64)
        lo_pad[: c.NPC] = lo_deg[order]
        hi_pad[: c.NPC] = hi_deg[order]
        Ls[ci] = lo_pad.reshape(c.G, P).max(1)
        Hs[ci] = hi_pad.reshape(c.G, P).max(1)
        inv = np.empty(c.NPC, np.int64)
        inv[order] = np.arange(c.NPC)
        core_data.append((inv[loc], s, hi, owned[order], lo_pad, hi_pad))

    S = Sched()
    S.L = Ls.max(0)
    S.H = Hs.max(0)
    S.D = S.L + S.H
    S.D = (S.D + 1) // 2 * 2  # slot counts multiple of 2 (pairwise pre-add)
    S.Doff = np.concatenate([[0], np.cumsum(S.D)])
    S.SUMD = int(S.D.sum())
    S.group_chunks = []
    for g in range(c.G):
        D = int(S.D[g])
        lst = []
        a = 0
        step = CMAX - (CMAX % 2)
        while D - a > 0:
            k = min(step, D - a)
            lst.append((a, k))
            a += k
        S.group_chunks.append(lst)
    S.CMAXG = int(max((k for lst in S.group_chunks for _, k in lst), default=1))

    per_core = []
    for ci in range(NC):
        pos, s, hi, owned_nodes, lo_pad, hi_pad = core_data[ci]
        # init every slot with the sentinel row of its half (padding slots
        # gather a zero-feature row whose el columns are SENT_EL)
        A = np.empty(S.SUMD * P, np.int16)
        for g in range(c.G):
            d0, L, D = int(S.Doff[g]), int(S.L[g]), int(S.D[g])
            A[d0 * P : (d0 + L) * P] = c.SENT_LO
            A[(d0 + L) * P : (d0 + D) * P] = c.SENT_HI
        for half, sel in (("lo", ~hi), ("hi", hi)):
            posh = pos[sel]
            rows = l1row[s[sel]]
            if half == "hi":
                rows = rows - c.HI_START
            o2 = np.argsort(posh, kind="stable")
            posh_s = posh[o2]
            rows_s = rows[o2]
            starts = np.searchsorted(posh_s, np.arange(c.NPAD), side="left")
            j = np.arange(len(posh_s)) - starts[posh_s]
            gg = posh_s // P
            part = posh_s % P
            d = j + (S.L[gg] if half == "hi" else 0)
            assert (d < S.D[gg]).all()
            assert rows_s.min(initial=0) >= 0 and rows_s.max(initial=0) < 32768
            A[(S.Doff[gg] + d) * P + part] = rows_s.astype(np.int16)
        segs = []
        for g in range(c.G):
            n = int(S.D[g]) * P
            if n == 0:
                continue
            off = int(S.Doff[g]) * P
            segs.append(A[off : off + n].reshape(n // 16, 16).T)
        w = np.concatenate(segs, axis=1) if segs else np.zeros((16, 0), A.dtype)
        idxb = np.ascontiguousarray(np.tile(w, (8, 1)))

        xs = np.zeros((c.NPAD, c.IN), np.float32)
        xs[: c.NPC] = x[owned_nodes]
        per_core.append({"x": _bf(xs), "idxb": idxb})
    orders = [cd[3] for cd in core_data]  # global node ids in rank order
    return S, per_core, orders


def _bf(a):
    import jax.numpy as jnp

    return np.asarray(jnp.asarray(a, dtype=jnp.bfloat16))


def make_weights(cfg, W1, al1, ar1, b1, W2, al2, ar2, b2):
    c = cfg
    W1 = W1.astype(np.float64)
    A = np.zeros((c.IN, c.IN), np.float64)
    for h in range(c.H1):
        M = np.stack([al1[h], ar1[h]]).astype(np.float64)
        Qf, _ = np.linalg.qr(M.T, mode="complete")
        Ah = np.concatenate([M, Qf[:, 2:].T], axis=0)
        A[h * c.D1 : (h + 1) * c.D1, h * c.D1 : (h + 1) * c.D1] = Ah
    Ainv = np.linalg.inv(A)
    w1g = (W1 @ A.T).astype(np.float32)
    ainvt = np.ascontiguousarray(Ainv.T.astype(np.float32))
    w2g = np.zeros((c.IN, c.C2), np.float32)
    w2g[:, : c.O2] = W2
    w2g[:, c.O2] = W2.astype(np.float64) @ al2[0].astype(np.float64)
    w2g[:, c.O2 + 1] = W2.astype(np.float64) @ ar2[0].astype(np.float64)
    b1rep = np.ascontiguousarray(np.tile(b1[None, :], (P, 1)).astype(np.float32))
    b2rep = np.ascontiguousarray(np.tile(b2[None, :], (P, 1)).astype(np.float32))
    # sentinel table rows: zero features, el columns = SENT_EL (pre-leaky)
    sent1 = np.zeros((1, c.IN), np.float32)
    sent1[0, 0 : c.IN : c.D1] = SENT_EL
    sent2 = np.zeros((1, c.C2), np.float32)
    sent2[0, c.O2] = SENT_EL
    # single packed bf16 constant tensor (fewer per-launch PJRT args):
    # rows [0:IN] = [w1g | ainvt | w2g]; row IN = [sent1 | sent2 | 0]
    wpack = np.zeros((c.IN + 1, 2 * c.IN + c.C2), np.float32)
    wpack[: c.IN, 0 : c.IN] = w1g
    wpack[: c.IN, c.IN : 2 * c.IN] = ainvt
    wpack[: c.IN, 2 * c.IN :] = w2g
    wpack[c.IN, 0 : c.IN] = sent1[0]
    wpack[c.IN, c.IN : c.IN + c.C2] = sent2[0]
    return {
        "wpack": _bf(wpack),
        "b1rep": b1rep,
        "b2rep": b2rep,
    }


def _chunk_segments(S, g, a, k):
    L = int(S.L[g])
    segs = []
    lo_n = max(0, min(a + k, L) - a)
    if lo_n:
        segs.append(("lo", a, lo_n))
    hi_a = max(a, L)
    hi_n = a + k - hi_a
    if hi_n > 0:
        segs.append(("hi", hi_a, hi_n))
    return segs


def build_program(cfg, S, probe=False, use_bias=True, fake_ag=False):
    c = cfg
    IN, C2, O2, H1, D1, G = c.IN, c.C2, c.O2, c.H1, c.D1, c.G
    AL = mybir.AluOpType
    AF = mybir.ActivationFunctionType
    AX = mybir.AxisListType
    CM = S.CMAXG

    ndev = 1 if probe else NC
    nc = bacc.Bacc("TRN2", target_bir_lowering=False, debug=False, num_devices=ndev, num_swdge_queues=4)
    xin = nc.dram_tensor("x", [c.NPAD, IN], BF16, kind="ExternalInput")
    idxb = nc.dram_tensor("idxb", [P, 8 * max(S.SUMD, 1)], I16, kind="ExternalInput")
    wpack_d = nc.dram_tensor("wpack", [IN + 1, 2 * IN + C2], BF16, kind="ExternalInput")
    w1g_d = wpack_d[0:IN, 0:IN]
    ainvt_d = wpack_d[0:IN, IN : 2 * IN]
    w2g_d = wpack_d[0:IN, 2 * IN : 2 * IN + C2]
    sent1_d = wpack_d[IN : IN + 1, 0:IN]
    sent2_d = wpack_d[IN : IN + 1, IN : IN + C2]
    if use_bias:
        b1_d = nc.dram_tensor("b1rep", [P, IN], F32, kind="ExternalInput")
        b2_d = nc.dram_tensor("b2rep", [P, O2], F32, kind="ExternalInput")
    outp = nc.dram_tensor("out", [c.NPAD, O2], F32, kind="ExternalOutput")

    with tile.TileContext(nc) as tc:
        with (
            tc.tile_pool(name="dram", bufs=1, space="DRAM") as dram,
            tc.tile_pool(name="const", bufs=1) as const,
        ):
            bounce1 = dram.tile([c.NPAD, IN], BF16)
            fs1 = dram.tile([NC * c.NPAD, IN], BF16, addr_space="Shared")
            bounce2 = dram.tile([c.NPAD, C2], BF16)
            fs2 = dram.tile([NC * c.NPAD, C2], BF16, addr_space="Shared")

            identb = const.tile([P, P], BF16)
            make_identity(nc, identb[:])
            w1g_a = const.tile([P, IN], BF16)
            w1g_b = const.tile([P, IN], BF16)
            nc.sync.dma_start(w1g_a[:], w1g_d[0:P, :])
            nc.sync.dma_start(w1g_b[:], w1g_d[P : 2 * P, :])
            ainvt_a = const.tile([P, IN], BF16)
            ainvt_b = const.tile([P, IN], BF16)
            nc.sync.dma_start(ainvt_a[:], ainvt_d[0:P, :])
            nc.sync.dma_start(ainvt_b[:], ainvt_d[P : 2 * P, :])
            w2g_a = const.tile([P, C2], BF16)
            w2g_b = const.tile([P, C2], BF16)
            nc.sync.dma_start(w2g_a[:], w2g_d[0:P, :])
            nc.sync.dma_start(w2g_b[:], w2g_d[P : 2 * P, :])
            if use_bias:
                b1_sb = const.tile([P, IN], F32)
                b2_sb = const.tile([P, O2], F32)
                nc.sync.dma_start(b1_sb[:], b1_d[:, :])
                nc.sync.dma_start(b2_sb[:], b2_d[:, :])
            er1tab = const.tile([P, G, H1], F32)
            er2tab = const.tile([P, G], F32)
            # whole idx blob resident in SBUF: gathers slice it directly
            idx_sb = const.tile([P, 8 * max(S.SUMD, 1)], I16)
            nc.sync.dma_start(idx_sb[:], idxb[:, :])

            # ---- projection: g-table = x @ w1g (bf16 in, fp32 psum) ----
            # processed in group pairs: one [P, 2*IN] load carries 256 rows
            # (partition p = rows 2p, 2p+1 of the pair-block), halving HWDGE
            # dispatches; outputs written back with the inverse interleave.
            with (
                nc.named_scope("proj"),
                tc.tile_pool(name="proj", bufs=4) as proj,
                tc.tile_pool(name="pp", bufs=2, space="PSUM") as pp,
            ):
                for t in range(0, G, 2):
                    pair = 2 if t + 1 < G else 1
                    W = pair * IN
                    xt = proj.tile([P, 2 * IN], BF16, tag="x")
                    if pair == 2:
                        # partition p carries row p of group t and row p of
                        # group t+1 side by side
                        nc.sync.dma_start(
                            xt[:].rearrange("p (c f) -> p c f", c=2),
                            xin[t * P : (t + 2) * P, :].rearrange(
                                "(c p) f -> p c f", c=2
                            ),
                        )
                    else:
                        nc.sync.dma_start(xt[:, 0:W], xin[t * P : (t + 1) * P, :])
                    ps_t = pp.tile([P, 2 * IN], BF16, tag="ps_t")
                    for blk in range(2 * pair):
                        nc.tensor.transpose(
                            ps_t[:, blk * P : (blk + 1) * P],
                            xt[:, blk * P : (blk + 1) * P],
                            identb[:],
                        )
                    xT = proj.tile([P, 2 * IN], BF16, tag="xT")
                    nc.vector.tensor_copy(xT[:, 0:W], ps_t[:, 0:W])
                    ps_g = pp.tile([P, 2 * IN], F32, tag="ps_g")
                    for q in range(pair):
                        nc.tensor.matmul(
                            out=ps_g[:, q * IN : (q + 1) * IN],
                            lhsT=xT[:, 2 * q * P : (2 * q + 1) * P],
                            rhs=w1g_a[:], start=True, stop=False,
                        )
                        nc.tensor.matmul(
                            out=ps_g[:, q * IN : (q + 1) * IN],
                            lhsT=xT[:, (2 * q + 1) * P : (2 * q + 2) * P],
                            rhs=w1g_b[:], start=False, stop=True,
                        )
                    gsb = proj.tile([P, 2 * IN], BF16, tag="gsb")
                    nc.vector.tensor_copy(gsb[:, 0:W], ps_g[:, 0:W])
                    for q in range(pair):
                        nc.vector.tensor_copy(
                            er1tab[:, t + q, :], ps_g[:, q * IN + 1 : (q + 1) * IN : D1]
                        )
                    if pair == 2:
                        nc.sync.dma_start(
                            bounce1[t * P : (t + 2) * P, :].rearrange(
                                "(c p) f -> p c f", c=2
                            ),
                            gsb[:].rearrange("p (c f) -> p c f", c=2),
                        )
                    else:
                        nc.sync.dma_start(
                            bounce1[t * P : (t + 1) * P, :], gsb[:, 0:W]
                        )
                # sentinel row for padding slots (lo half uses core 0's copy,
                # hi half core 7's; every core writes its own local row NPC)
                nc.sync.dma_start(bounce1[c.NPC : c.NPC + 1, :], sent1_d[0:1, :])

            with nc.named_scope("ag1"):
                if probe or fake_ag:
                    nc.sync.dma_start(fs1[0 : c.NPAD, :], bounce1[:, :])
                else:
                    nc.gpsimd.collective_compute(
                        "AllGather", mybir.AluOpType.bypass,
                        replica_groups=[list(range(NC))],
                        ins=[bounce1.opt()], outs=[fs1.opt()],
                    )

            qctr = [0]
            # ---- layer-1 edge phase ----
            with (
                nc.named_scope("edge1"),
                tc.tile_pool(name="edge", bufs=5) as ep,
                tc.tile_pool(name="wide", bufs=3) as wp,
                tc.tile_pool(name="msgp", bufs=2) as mp,
                tc.tile_pool(name="acc", bufs=3) as ac,
                tc.tile_pool(name="pp2", bufs=2, space="PSUM") as pp2,
            ):
                # 1-stage software pipeline: iteration g emits group g's
                # gather/attention/message/PE-accumulate (head) and group
                # g-1's PE-dependent tail (copy/rotate/normalize/relu/W2), so
                # no engine queue has later head work parked behind a
                # cross-engine wait from the previous group's tail.
                prev = None
                for g in range(G + 1):
                  if g < G:
                    red = nc.vector
                    denom = ac.tile([P, H1], F32, tag="denom")
                    # segment-sum accumulates in PSUM via per-slot transpose
                    # matmuls: ps_gt[f, p] += msg[p, j, f]^T (f32, exact)
                    ps_gt = pp2.tile([P, IN], F32, tag="ps_gt")
                    chunks = S.group_chunks[g]
                    assert chunks, "every group must have slots"
                    first = True
                    for ci_ch, (a, k) in enumerate(chunks):
                        last_ch = ci_ch == len(chunks) - 1
                        moff = int(S.Doff[g]) + a
                        gb = ep.tile([P, CM, IN], BF16, tag="gb")
                        for half, s0, n in _chunk_segments(S, g, a, k):
                            src_ap = (
                                fs1[0 : c.LO_END, :]
                                if half == "lo"
                                else fs1[c.HI_START : c.NROWS, :]
                            )
                            o = s0 - a
                            nc.gpsimd.dma_gather(
                                gb[:, o : o + n, :], src_ap,
                                idx_sb[:, 8 * (moff + o) : 8 * (moff + o + n)],
                                P * n, P * n, IN, single_packet=False,
                                queue_num=qctr[0] % 4,
                            )
                            qctr[0] += 1
                        # e1 = el_src + er_dst; leaky relu on DVE (padding
                        # slots carry el=SENT_EL -> exp ~ 0)
                        e1 = ep.tile([P, CM, H1], F32, tag="e1")
                        nc.vector.tensor_tensor(
                            out=e1[:, 0:k, :],
                            in0=gb[:, 0:k, 0 : IN : D1],
                            in1=er1tab[:, g, None, :].to_broadcast([P, k, H1]),
                            op=AL.add,
                        )
                        e2 = ep.tile([P, CM, H1], F32, tag="e2")
                        nc.vector.scalar_tensor_tensor(
                            out=e2[:, 0:k, :], in0=e1[:, 0:k, :], scalar=NEG,
                            in1=e1[:, 0:k, :], op0=AL.mult, op1=AL.max,
                        )
                        # narrow widened exp: [P, k, H1, WE] bf16 on Activation
                        e3w = wp.tile([P, CM, H1, WE], BF16, tag="e3w")
                        nc.scalar.activation(
                            e3w[:, 0:k, :, :],
                            e2[:, 0:k, :, None].to_broadcast([P, k, H1, WE]),
                            AF.Exp,
                        )
                        # denom: reduce exp at col 0 of each head block
                        if first:
                            red.tensor_reduce(
                                out=denom[:, :],
                                in_=e3w[:, 0:k, :, 0].rearrange(
                                    "p j h -> p h j"
                                ),
                                axis=AX.X, op=AL.add,
                            )
                        else:
                            dtmp = ep.tile([P, H1], F32, tag="dtmp")
                            red.tensor_reduce(
                                out=dtmp[:, :],
                                in_=e3w[:, 0:k, :, 0].rearrange(
                                    "p j h -> p h j"
                                ),
                                axis=AX.X, op=AL.add,
                            )
                            nc.vector.tensor_tensor(
                                out=denom[:, :], in0=denom[:, :], in1=dtmp[:, :], op=AL.add
                            )
                        # packed bf16 message multiply (DVE 2x mode); the WE-wide
                        # exp block is replicated across each head's D1 columns
                        # by a stride-0 AP dim, keeping the last dim packed
                        msg = mp.tile([P, CM, IN], BF16, tag="msg")
                        nc.vector.tensor_tensor(
                            out=msg[:, 0:k, :].rearrange(
                                "p j (h r w) -> p j h r w", h=H1, w=WE
                            ),
                            in0=gb[:, 0:k, :].rearrange(
                                "p j (h r w) -> p j h r w", h=H1, w=WE
                            ),
                            in1=e3w[:, 0:k, :, None, :].to_broadcast(
                                [P, k, H1, D1 // WE, WE]
                            ),
                            op=AL.mult,
                        )
                        # segment sum on the TensorEngine: per-slot transpose
                        # matmuls accumulate msg^T into PSUM (f32 adds); the
                        # two feature-block chains run back to back so each
                        # start..stop accumulation run is contiguous
                        for blk in range(2):
                            for j in range(k):
                                nc.tensor.matmul(
                                    out=ps_gt[:, blk * P : (blk + 1) * P],
                                    lhsT=msg[:, j, blk * P : (blk + 1) * P],
                                    rhs=identb[:],
                                    start=first and j == 0,
                                    stop=last_ch and j == k - 1,
                                )
                        first = False
                    rden = ac.tile([P, H1], F32, tag="rden")
                    nc.vector.reciprocal(rden[:, :], denom[:, :])
                    cur = (ps_gt, rden)
                  if g > 0:
                    ps_gt_p, rden_p = prev
                    gp = g - 1
                    gT = ac.tile([P, IN], BF16, tag="gT")
                    nc.scalar.copy(gT[:], ps_gt_p[:])
                    ps_f = pp2.tile([P, IN], F32, tag="ps_f")
                    nc.tensor.matmul(
                        out=ps_f[:], lhsT=gT[:, 0:P], rhs=ainvt_a[:],
                        start=True, stop=False,
                    )
                    nc.tensor.matmul(
                        out=ps_f[:], lhsT=gT[:, P : 2 * P], rhs=ainvt_b[:],
                        start=False, stop=True,
                    )
                    # normalization commutes past the block-diagonal rotation:
                    # apply 1/denom per (node, head) after the matmul. Runs on
                    # the Activation engine (per-partition scale) so the wait
                    # on the PE chain does not block the in-order DVE queue
                    # ahead of the next group's attention/message ops.
                    hn = ac.tile([P, IN], F32, tag="hn")
                    for h in range(H1):
                        nc.scalar.mul(
                            hn[:, h * D1 : (h + 1) * D1],
                            ps_f[:, h * D1 : (h + 1) * D1],
                            rden_p[:, h : h + 1],
                        )
                    hr = ac.tile([P, IN], BF16, tag="hr")
                    if use_bias:
                        hq = ac.tile([P, IN], F32, tag="hq")
                        nc.vector.tensor_add(hq[:], hn[:], b1_sb[:])
                        nc.scalar.activation(hr[:], hq[:], AF.Relu)
                    else:
                        nc.scalar.activation(hr[:], hn[:], AF.Relu)
                    ps_b = pp2.tile([P, IN], BF16, tag="ps_a")
                    nc.tensor.transpose(ps_b[:, 0:P], hr[:, 0:P], identb[:])
                    nc.tensor.transpose(ps_b[:, P : 2 * P], hr[:, P : 2 * P], identb[:])
                    hT = ac.tile([P, IN], BF16, tag="hT")
                    nc.scalar.copy(hT[:], ps_b[:])
                    ps_2 = pp2.tile([P, C2], F32, tag="ps_2")
                    nc.tensor.matmul(
                        out=ps_2[:], lhsT=hT[:, 0:P], rhs=w2g_a[:],
                        start=True, stop=False,
                    )
                    nc.tensor.matmul(
                        out=ps_2[:], lhsT=hT[:, P : 2 * P], rhs=w2g_b[:],
                        start=False, stop=True,
                    )
                    f2 = ac.tile([P, C2], BF16, tag="f2")
                    nc.scalar.copy(f2[:], ps_2[:])
                    nc.scalar.copy(
                        er2tab[:, gp : gp + 1], ps_2[:, O2 + 1 : O2 + 2]
                    )
                    nc.sync.dma_start(bounce2[gp * P : (gp + 1) * P, :], f2[:])
                  if g < G:
                    prev = cur
                nc.sync.dma_start(bounce2[c.NPC : c.NPC + 1, :], sent2_d[0:1, :])

            with nc.named_scope("ag2"):
                if probe or fake_ag:
                    nc.sync.dma_start(fs2[0 : c.NPAD, :], bounce2[:, :])
                else:
                    nc.gpsimd.collective_compute(
                        "AllGather", mybir.AluOpType.bypass,
                        replica_groups=[list(range(NC))],
                        ins=[bounce2.opt()], outs=[fs2.opt()],
                    )

            # ---- layer-2 edge phase ----
            with (
                nc.named_scope("edge2"),
                tc.tile_pool(name="edge2", bufs=5) as ep,
                tc.tile_pool(name="wide2", bufs=3) as wp,
                tc.tile_pool(name="msgp2", bufs=2) as mp,
                tc.tile_pool(name="acc2", bufs=3) as ac,
            ):
                for g in range(G):
                    red = nc.vector
                    denom = ac.tile([P, 1], F32, tag="denom")
                    out2 = ac.tile([P, O2], F32, tag="out2")
                    first = True
                    for a, k in S.group_chunks[g]:
                        moff = int(S.Doff[g]) + a
                        gb = ep.tile([P, CM, C2], BF16, tag="gb")
                        for half, s0, n in _chunk_segments(S, g, a, k):
                            src_ap = (
                                fs2[0 : c.LO_END, :]
                                if half == "lo"
                                else fs2[c.HI_START : c.NROWS, :]
                            )
                            o = s0 - a
                            nc.gpsimd.dma_gather(
                                gb[:, o : o + n, :], src_ap,
                                idx_sb[:, 8 * (moff + o) : 8 * (moff + o + n)],
                                P * n, P * n, C2, single_packet=False,
                                queue_num=qctr[0] % 4,
                            )
                            qctr[0] += 1
                        e1 = ep.tile([P, CM], F32, tag="e1")
                        nc.vector.tensor_tensor(
                            out=e1[:, 0:k],
                            in0=gb[:, 0:k, O2],
                            in1=er2tab[:, g : g + 1].to_broadcast([P, k]),
                            op=AL.add,
                        )
                        e2 = ep.tile([P, CM], F32, tag="e2")
                        nc.vector.scalar_tensor_tensor(
                            out=e2[:, 0:k], in0=e1[:, 0:k], scalar=NEG,
                            in1=e1[:, 0:k], op0=AL.mult, op1=AL.max,
                        )
                        # widened exp [P, k, O2+1] bf16 (col O2 doubles as denom src)
                        e3w = wp.tile([P, CM, O2 + 1], BF16, tag="e3w")
                        nc.scalar.activation(
                            e3w[:, 0:k, :],
                            e2[:, 0:k, None].to_broadcast([P, k, O2 + 1]),
                            AF.Exp,
                        )
                        if first:
                            red.tensor_reduce(
                                out=denom[:, :],
                                in_=e3w[:, 0:k, O2 : O2 + 1].rearrange(
                                    "p j f -> p f j"
                                ),
                                axis=AX.X, op=AL.add,
                            )
                        else:
                            dtmp = ep.tile([P, 1], F32, tag="dtmp")
                            red.tensor_reduce(
                                out=dtmp[:, :],
                                in_=e3w[:, 0:k, O2 : O2 + 1].rearrange(
                                    "p j f -> p f j"
                                ),
                                axis=AX.X, op=AL.add,
                            )
                            nc.vector.tensor_tensor(
                                out=denom[:, :], in0=denom[:, :], in1=dtmp[:, :], op=AL.add
                            )
                        msg = mp.tile([P, CM, O2], BF16, tag="msg")
                        nc.vector.tensor_tensor(
                            out=msg[:, 0:k, :],
                            in0=gb[:, 0:k, 0:O2],
                            in1=e3w[:, 0:k, 0:O2],
                            op=AL.mult,
                        )
                        k2 = k // 2
                        k4 = k2 // 2
                        pre = mp.tile([P, CM // 2, O2], BF16, tag="pre")
                        nc.vector.tensor_tensor(
                            out=pre[:, 0:k2, :],
                            in0=msg[:, 0 : 2 * k2 : 2, :],
                            in1=msg[:, 1 : 2 * k2 : 2, :],
                            op=AL.add,
                        )
                        pre2 = mp.tile([P, CM // 4 + 1, O2], BF16, tag="pre2")
                        nc.vector.tensor_tensor(
                            out=pre2[:, 0:k4, :],
                            in0=pre[:, 0 : 2 * k4 : 2, :],
                            in1=pre[:, 1 : 2 * k4 : 2, :],
                            op=AL.add,
                        )
                        if first:
                            red.tensor_reduce(
                                out=out2[:, :],
                                in_=pre2[:, 0:k4, :].rearrange("p j f -> p f j"),
                                axis=AX.X, op=AL.add,
                            )
                        else:
                            otmp = ep.tile([P, O2], F32, tag="otmp")
                            red.tensor_reduce(
                                out=otmp[:, :],
                                in_=pre2[:, 0:k4, :].rearrange("p j f -> p f j"),
                                axis=AX.X, op=AL.add,
                            )
                            nc.vector.tensor_tensor(
                                out=out2[:, :], in0=out2[:, :], in1=otmp[:, :], op=AL.add
                            )
                        if k2 % 2:
                            nc.vector.tensor_tensor(
                                out=out2[:, :], in0=out2[:, :],
                                in1=pre[:, k2 - 1, :], op=AL.add,
                            )
                        first = False
                    if first:
                        nc.vector.memset(denom[:, :], 1.0)
                        nc.vector.memset(out2[:, :], 0.0)
                    rden = ac.tile([P, 1], F32, tag="rden")
                    nc.vector.reciprocal(rden[:, :], denom[:, :])
                    on = ac.tile([P, O2], F32, tag="on")
                    nc.vector.tensor_scalar_mul(on[:], out2[:], rden[:, 0:1])
                    if use_bias:
                        ob = ac.tile([P, O2], F32, tag="ob")
                        nc.vector.tensor_add(ob[:], on[:], b2_sb[:])
                        nc.sync.dma_start(outp[g * P : (g + 1) * P, :], ob[:])
                    else:
                        nc.sync.dma_start(outp[g * P : (g + 1) * P, :], on[:])
    nc.compile()
    return nc



# ---------------------------------------------------------------- runner with
# persistent executable (avoids re-jit on repeated kernel() calls)
import os as _os
_os.environ.setdefault("JAX_COMPILATION_CACHE_DIR", "/tmp/jax_neff_cache")

N_NODES = 50000
N_EDGES = 800000

_CACHE = {}


class _Runner:
    def __init__(self, cfg, S, use_bias=True, fake_ag=False):
        import jax
        from jax.sharding import Mesh, PartitionSpec
        from jax.experimental.shard_map import shard_map
        import concourse.bass2jax as b2j

        self.cfg = cfg
        nc = build_program(cfg, S, use_bias=use_bias, fake_ag=fake_ag)
        b2j.install_neuronx_cc_hook()
        partition_name = (
            nc.partition_id_tensor.name if nc.partition_id_tensor else None
        )
        in_names, out_names, out_avals, zero_outs = [], [], [], []
        for alloc in nc.m.functions[0].allocations:
            if not isinstance(alloc, mybir.MemoryLocationSet):
                continue
            name = alloc.memorylocations[0].name
            if alloc.kind == "ExternalInput":
                if name != partition_name:
                    in_names.append(name)
            elif alloc.kind == "ExternalOutput":
                out_names.append(name)
                shape = tuple(alloc.tensor_shape)
                dtype = mybir.dt.np(alloc.dtype)
                out_avals.append(jax.core.ShapedArray(shape, dtype))
                zero_outs.append(np.zeros(shape, dtype))
        self.n_params = len(in_names)
        self.param_names = list(in_names)
        self.out_names = out_names
        self.zero_outs = zero_outs
        all_in = in_names + out_names
        if partition_name is not None:
            all_in.append(partition_name)

        def _body(*args):
            operands = list(args)
            if partition_name is not None:
                operands.append(b2j.partition_id_tensor())
            outs = b2j._bass_exec_p.bind(
                *operands,
                out_avals=tuple(out_avals),
                in_names=tuple(all_in),
                out_names=tuple(out_names),
                lowering_input_output_aliases=(),
                sim_require_finite=True,
                sim_require_nnan=True,
                nc=nc,
            )
            return tuple(outs)

        devices = jax.devices()[:NC]
        assert len(devices) == NC
        self.mesh = Mesh(np.asarray(devices), ("core",))
        n_io = self.n_params + len(out_names)
        self.fn = jax.jit(
            shard_map(
                _body,
                mesh=self.mesh,
                in_specs=(PartitionSpec("core"),) * n_io,
                out_specs=(PartitionSpec("core"),) * len(out_names),
                check_rep=False,
            ),
            keep_unused=True,
        )
        self.jax = jax
        self.PartitionSpec = PartitionSpec

    def put_inputs(self, in_maps):
        jax = self.jax
        from jax.sharding import NamedSharding

        sh = NamedSharding(self.mesh, self.PartitionSpec("core"))
        args = []
        for name in self.param_names:
            g = np.concatenate([np.asarray(m[name]) for m in in_maps], axis=0)
            args.append(jax.device_put(g, sh))
        for z in self.zero_outs:
            g = np.zeros((NC * z.shape[0], *z.shape[1:]), z.dtype)
            args.append(jax.device_put(g, sh))
        return args

    def __call__(self, args):
        outs = self.fn(*args)
        return [np.asarray(o) for o in outs]


def _get_runner(cfg, S, key, use_bias=True):
    r = _CACHE.get(key)
    if r is None:
        r = _Runner(cfg, S, use_bias=use_bias)
        _CACHE[key] = r
    return r


def _prepare(inputs):
    cfg = Cfg(N_NODES, N_EDGES)
    x = np.ascontiguousarray(np.asarray(inputs["x"], np.float32))
    src = np.asarray(inputs["src"])
    dst = np.asarray(inputs["dst"])
    S, per_core, orders = preprocess(cfg, x, src, dst)
    shared = make_weights(
        cfg,
        *(np.asarray(inputs[k], np.float32)
          for k in ("W1", "al1", "ar1", "b1", "W2", "al2", "ar2", "b2")),
    )
    in_maps = [dict(shared, **pc) for pc in per_core]
    import hashlib

    use_bias = bool(
        np.asarray(inputs["b1"]).any() or np.asarray(inputs["b2"]).any()
    )
    key = (
        hashlib.sha1(src.tobytes() + dst.tobytes()).hexdigest(),
        use_bias,
    )
    return cfg, S, in_maps, orders, key


def kernel(**inputs) -> np.ndarray:
    cfg, S, in_maps, orders, key = _prepare(inputs)
    runner = _get_runner(cfg, S, key, use_bias=key[1])
    args = runner.put_inputs(in_maps)
    outs = runner(args)
    full = outs[runner.out_names.index("out")].reshape(NC, cfg.NPAD, cfg.O2)
    out = np.empty((cfg.N, cfg.O2), np.float32)
    for ci in range(NC):
        out[orders[ci]] = full[ci, : cfg.NPC]
    return out

